# revision 2
# baseline (speedup 1.0000x reference)
"""FastRNN Trainium2 kernel: h_t = sigmoid(beta)*h_{t-1} + sigmoid(alpha)*tanh(x_t@W + h_{t-1}@U + b).

Strategy (data-parallel over batch, 8 NeuronCores, 8 sequences each):
  - Precompute wxb[tok,h] = x@W + bias on the PE (fp32r matmuls; x tiles are
    PE-transposed on device to form the stationary operand).
  - 512-step recurrence: 5 fp32r matmuls per step accumulate
    wxb_t + h_{t-1}@U into PSUM (the wxb_t term is injected via a selector
    matmul so K=128 everywhere); ScalarE tanh; PE transposes of tanh output;
    one VectorE op forms h_t = sb*h_{t-1} + sa*c_t directly in a transposed
    history buffer that both feeds the next step's matmuls (as stationary
    weights) and IS the output (DMA'd out in chunks, host does a pure layout
    permute when unsharding).
"""

from contextlib import ExitStack

import numpy as np

import concourse.bass as bass
import concourse.mybir as mybir
from concourse.bass_utils import run_bass_kernel_spmd
from concourse.tile import TileContext
from concourse.vector_clock import ScopedClock

F32 = mybir.dt.float32
F32R = mybir.dt.float32r
AF = mybir.ActivationFunctionType
ALU = mybir.AluOpType

B, T, I, H = 64, 512, 256, 512
N_CORES = 8
BC = B // N_CORES
KT = H // 128
TT = T // 16


class PatchedTileContext(TileContext):
    """The stock tail drain can carry more sem waits than this walrus's
    CTRL-instruction wait slots; spill the excess onto preceding NOPs."""

    def _drain_and_barrier(self, tick_clock, wait_clock):
        nc = self.nc
        drain_inst = nc.sync.drain()
        wait_clock.add_sem_waits(
            drain_inst.ins, ScopedClock({None: tick_clock.global_clock})
        )
        si = drain_inst.ins.sync_info
        waits = list(si.on_wait or []) if si is not None else []
        if len(waits) > 1:
            bb = nc.cur_bb.bb
            idx = bb.instructions.index(drain_inst.ins)
            extra, keep = waits[:-1], waits[-1:]
            si.on_wait = keep
            for i in range(len(extra)):
                nop = nc.sync.nop()
                nsi = nop.ins.sync_info
                if nsi is None:
                    nop.ins.sync_info = mybir.SyncInfo(
                        on_wait=extra[i : i + 1], on_update=[]
                    )
                else:
                    nsi.on_wait = extra[i : i + 1]
                bb.instructions.remove(nop.ins)
                bb.instructions.insert(idx, nop.ins)
                idx += 1
        nc.all_engine_barrier()
        popped = nc._tile_sem_poison_stack.pop()
        assert popped is self._sem_poison
        nc.clear_and_free_semaphores(list(self.sems.allocated().values()))
        nc.all_engine_barrier()


_CTRL_TYPES = ("InstDrain", "InstNop", "InstEventSemOp")


def spill_waits(nc, compute_limit=1, ctrl_limit=1):
    """Move excess per-instruction sync waits onto preceding same-engine NOPs
    (this walrus accepts at most one wait slot per instruction)."""
    for f in nc.m.functions:
        for bb in f.blocks:
            insts = list(bb.instructions)
            for inst in insts:
                si = inst.sync_info
                if si is None or not si.on_wait:
                    continue
                limit = (
                    ctrl_limit
                    if type(inst).__name__ in _CTRL_TYPES
                    else compute_limit
                )
                waits = list(si.on_wait)
                if len(waits) <= limit:
                    continue
                keep = waits[-limit:]
                extra = waits[:-limit]
                si.on_wait = keep
                idx = bb.instructions.index(inst)
                for i in range(0, len(extra), ctrl_limit):
                    nop = nc.engines[inst.engine].nop()
                    nsi = nop.ins.sync_info
                    chunk = extra[i : i + ctrl_limit]
                    if nsi is None:
                        nop.ins.sync_info = mybir.SyncInfo(
                            on_wait=chunk, on_update=[]
                        )
                    else:
                        nsi.on_wait = chunk
                    for f2 in nc.m.functions:
                        for bb2 in f2.blocks:
                            if nop.ins in bb2.instructions:
                                bb2.instructions.remove(nop.ins)
                    bb.instructions.insert(idx, nop.ins)
                    idx += 1


def build_nc(sa: float, sb: float):
    nc = bass.Bass(
        "TRN2", target_bir_lowering=False, debug=False, num_devices=N_CORES
    )

    x = nc.dram_tensor("x", [BC, T, I], F32, kind="ExternalInput")
    up = nc.dram_tensor("up", [H, H], F32, kind="ExternalInput")
    w = nc.dram_tensor("w", [I, H], F32, kind="ExternalInput")
    biasb = nc.dram_tensor("biasb", [128, H], F32, kind="ExternalInput")
    sel = nc.dram_tensor("sel", [128, 16 * BC], F32, kind="ExternalInput")
    id128 = nc.dram_tensor("id128", [128, 128], F32, kind="ExternalInput")
    id8 = nc.dram_tensor("id8", [BC, BC], F32, kind="ExternalInput")
    z0 = nc.dram_tensor("z0", [128, KT * BC], F32, kind="ExternalInput")
    out = nc.dram_tensor("out", [128, KT, T, BC], F32, kind="ExternalOutput")

    with PatchedTileContext(nc) as tc, ExitStack() as ctx:
        pool = lambda **kw: ctx.enter_context(tc.tile_pool(**kw))
        const = pool(name="const", bufs=1)
        u_sb = const.tile([128, KT, H], F32R)
        w_sb = const.tile([128, 2, H], F32R)
        biasb_sb = const.tile([128, H], F32)
        sel_sb = const.tile([128, 16, BC], F32R)
        id128_sb = const.tile([128, 128], F32R)
        id8_sb = const.tile([BC, BC], F32)
        wxb_sb = const.tile([128, TT, H], F32R)
        gbuf = const.tile([128, KT, T + 1, BC], F32R)

        for k in range(KT):
            nc.gpsimd.dma_start(
                out=u_sb[:, k, :], in_=up[k * 128 : (k + 1) * 128, :]
            )
        for j in range(2):
            nc.gpsimd.dma_start(
                out=w_sb[:, j, :], in_=w[j * 128 : (j + 1) * 128, :]
            )
        nc.sync.dma_start(out=biasb_sb[:], in_=biasb[:])
        nc.gpsimd.dma_start(
            out=sel_sb[:], in_=sel[:].rearrange("p (t b) -> p t b", b=BC)
        )
        nc.gpsimd.dma_start(out=id128_sb[:], in_=id128[:])
        nc.sync.dma_start(out=id8_sb[:], in_=id8[:])
        nc.gpsimd.dma_start(
            out=gbuf[:, :, 0, :],
            in_=z0[:].rearrange("p (k b) -> p k b", b=BC),
        )

        # ---- wxb = x @ W + bias, tokens b-major within 16-step tiles ----
        xpool = pool(name="xp", bufs=3)
        xtpool = pool(name="xtp", bufs=3)
        psum = pool(name="ps", bufs=2, space="PSUM")
        for i in range(TT):
            t0 = i * 16
            xt = xpool.tile([128, I], F32R)
            for b in range(BC):
                nc.gpsimd.dma_start(
                    out=xt[b * 16 : (b + 1) * 16, :], in_=x[b, t0 : t0 + 16, :]
                )
            xT = xtpool.tile([128, 2, 128], F32R)
            for j in range(2):
                tp = psum.tile([128, 128], F32R, tag="tp")
                nc.tensor.transpose(
                    tp[:], xt[:, j * 128 : (j + 1) * 128], id128_sb[:]
                )
                nc.vector.tensor_copy(xT[:, j, :], tp[:])
            ps = psum.tile([128, H], F32, tag="mm")
            for j in range(2):
                nc.tensor.matmul(
                    ps[:], xT[:, j, :], w_sb[:, j, :],
                    start=(j == 0), stop=(j == 1),
                )
            nc.vector.scalar_tensor_tensor(
                out=wxb_sb[:, i, :], in0=ps[:], scalar=1.0, in1=biasb_sb[:],
                op0=ALU.bypass, op1=ALU.add,
            )

        # ---- recurrence ----
        cpool = pool(name="cp", bufs=3)
        tmppool = pool(name="tmp", bufs=3)
        for t in range(T):
            ti, tsub = t // 16, t % 16
            ps = psum.tile([BC, H], F32, tag="mm")
            nc.tensor.matmul(
                ps[:], sel_sb[:, tsub, :], wxb_sb[:, ti, :],
                start=True, stop=False,
            )
            for k in range(KT):
                nc.tensor.matmul(
                    ps[:], gbuf[:, k, t, :], u_sb[:, k, :],
                    start=False, stop=(k == KT - 1),
                )
            c_sb = cpool.tile([BC, H], F32)
            nc.scalar.activation(c_sb[:], ps[:], AF.Tanh)
            cT = psum.tile([128, KT, BC], F32, tag="ct")
            for j in range(KT):
                nc.tensor.transpose(
                    cT[:, j, :], c_sb[:, j * 128 : (j + 1) * 128], id8_sb[:]
                )
            tmp = tmppool.tile([128, KT, BC], F32)
            nc.vector.tensor_scalar_mul(tmp[:], gbuf[:, :, t, :], float(sb))
            nc.vector.scalar_tensor_tensor(
                out=gbuf[:, :, t + 1, :], in0=cT[:, :, :], scalar=float(sa),
                in1=tmp[:], op0=ALU.mult, op1=ALU.add,
            )

        # ---- output DMA (raw transposed history; host permutes) ----
        for c in range(TT):
            t0 = c * 16
            nc.sync.dma_start(
                out=out[:, :, t0 : t0 + 16, :],
                in_=gbuf[:, :, 1 + t0 : 1 + t0 + 16, :].bitcast(F32),
            )

    spill_waits(nc, compute_limit=1)
    return nc


_CACHE = {}


def bench_nc_and_inputs(inputs):
    """(nc, in_maps) for the timing harness — mirrors kernel()'s setup."""
    x = np.ascontiguousarray(np.asarray(inputs["x"], np.float32))
    W = np.ascontiguousarray(np.asarray(inputs["W"], np.float32))
    U = np.ascontiguousarray(np.asarray(inputs["U"], np.float32))
    bias = np.asarray(inputs["bias"], np.float32)
    sa = float(1.0 / (1.0 + np.exp(-np.float64(np.asarray(inputs["alpha"]).reshape(-1)[0]))))
    sb = float(1.0 / (1.0 + np.exp(-np.float64(np.asarray(inputs["beta"]).reshape(-1)[0]))))
    key = (sa, sb)
    if key not in _CACHE:
        _CACHE[key] = build_nc(sa, sb)
    nc = _CACHE[key]
    biasb = np.tile(bias.reshape(1, H), (128, 1))
    sel = np.zeros((128, 16 * BC), np.float32)
    for tsub in range(16):
        for b in range(BC):
            sel[b * 16 + tsub, tsub * BC + b] = 1.0
    in_maps = []
    for c in range(N_CORES):
        in_maps.append({
            "x": np.ascontiguousarray(x[c * BC : (c + 1) * BC]),
            "up": U, "w": W, "biasb": biasb, "sel": sel,
            "id128": np.eye(128, dtype=np.float32),
            "id8": np.eye(BC, dtype=np.float32),
            "z0": np.zeros((128, KT * BC), np.float32),
        })
    return nc, in_maps


def kernel(x, W, U, bias, alpha, beta):
    x = np.ascontiguousarray(np.asarray(x, np.float32))
    W = np.ascontiguousarray(np.asarray(W, np.float32))
    U = np.ascontiguousarray(np.asarray(U, np.float32))
    bias = np.asarray(bias, np.float32)
    sa = float(1.0 / (1.0 + np.exp(-np.float64(np.asarray(alpha).reshape(-1)[0]))))
    sb = float(1.0 / (1.0 + np.exp(-np.float64(np.asarray(beta).reshape(-1)[0]))))

    key = (sa, sb)
    if key not in _CACHE:
        _CACHE[key] = build_nc(sa, sb)
    nc = _CACHE[key]

    biasb = np.tile(bias.reshape(1, H), (128, 1))
    sel = np.zeros((128, 16 * BC), np.float32)
    for tsub in range(16):
        for b in range(BC):
            sel[b * 16 + tsub, tsub * BC + b] = 1.0
    id128 = np.eye(128, dtype=np.float32)
    id8 = np.eye(BC, dtype=np.float32)
    z0 = np.zeros((128, KT * BC), np.float32)

    in_maps = []
    for c in range(N_CORES):
        in_maps.append({
            "x": np.ascontiguousarray(x[c * BC : (c + 1) * BC]),
            "up": U, "w": W, "biasb": biasb, "sel": sel,
            "id128": id128, "id8": id8, "z0": z0,
        })

    res = run_bass_kernel_spmd(nc, in_maps, list(range(N_CORES))).results

    out = np.empty((B, T, H), np.float32)
    for c in range(N_CORES):
        # out_raw[p, k, t, b] = h_t[b, k*128+p]  -> pure layout permute
        out[c * BC : (c + 1) * BC] = (
            res[c]["out"].transpose(3, 2, 1, 0).reshape(BC, T, H)
        )
    return out

